# revision 37
# baseline (speedup 1.0000x reference)
"""MoE routing kernel for Trainium2 (8 NeuronCores, expert-parallel, bf16).

Problem: top-2-of-8 expert MLP with squared-ReLU, d_model=1024, d_ff=1024,
N=8192 tokens. The router (softmax + top-2, ~0.2% of FLOPs) runs on host in
float64; tokens are dispatched on host (gather + sqrt(combine-weight)
scaling - relu(sqrt(w)*z)^2 == w*relu(z)^2, so the combine weight folds
into the input and the device kernel is a plain 2-layer MLP). All device
data is bf16 (PSUM accumulation stays fp32): same 1 cycle/row PE rate as
f32r but half the DMA/SBUF, and full rate at any tile width.

Load balance: experts are sorted by token count and paired big-with-small;
each pair is served by two cores, each taking half of both experts'
tokens. Capacities (CA for the big slot, CB for the small slot) are the
max over pairs, shared by all cores so one SPMD program fits. Host
scatter-adds the per-core outputs.
"""

import sys

if "/opt/trn_rl_repo" not in sys.path:
    sys.path.insert(0, "/opt/trn_rl_repo")

import ml_dtypes
import numpy as np

import bass_rust
import concourse.bass as bass
import concourse.tile as tile
import concourse.tile_utils as tile_utils
from concourse import mybir
from concourse.bass_utils import run_bass_kernel_spmd
from concourse.vector_clock import ScopedClock

NUM_EXPERTS = 8
TOP_K = 2
D_MODEL = 1024
D_FF = 1024
N_CORES = 8
KC = D_MODEL // 128
FT = D_FF // 128
DT = D_MODEL // 128

BF16 = mybir.dt.bfloat16
F32 = mybir.dt.float32
NPBF16 = ml_dtypes.bfloat16

# Cayman has 208 KiB/partition usable; the stock constant leaves 16 KiB idle.
tile_utils.max_sbuf_usage = 208 * 1024

# ---------------------------------------------------------------------------
# Compat: this container's walrus rejects instructions carrying more than one
# sem wait ("Too many sync wait commands"). Replace the TileContext final
# drain with single-wait SP nops, and post-process the module so every
# instruction carries at most one (monotonic) wait.
# ---------------------------------------------------------------------------


def _patched_drain_and_barrier(self, tick_clock, wait_clock):
    probe = self.nc.sync.nop(nofuse=True)
    wait_clock.add_sem_waits(probe.ins, ScopedClock({None: tick_clock.global_clock}))
    si = probe.ins.sync_info
    waits = list(si.on_wait) if si is not None else []
    updates = list(si.on_update) if si is not None else []
    if len(waits) > 1:
        probe.ins.sync_info = bass_rust.SyncInfo(on_wait=[waits[0]], on_update=updates)
        for w in waits[1:]:
            extra = self.nc.sync.nop(nofuse=True)
            extra.ins.sync_info = bass_rust.SyncInfo(on_wait=[w], on_update=[])
    self.nc.sync.drain()
    assert self.sems is not None
    popped = self.nc._tile_sem_poison_stack.pop()
    assert popped is self._sem_poison
    # Device-side sem cleanup (dma_reset + sem_clear + trailing barrier)
    # costs ~4 us of NRT-expanded epilogue. Each kernel() call loads a
    # fresh NEFF (sems re-initialized at load), so only the host-side
    # bookkeeping of clear_and_free_semaphores is needed.
    sem_nums = [s.num for s in self.sems.allocated().values()]
    self.nc._state.prepend_free_semaphores(sem_nums)
    for poison_set in self.nc._tile_sem_poison_stack:
        poison_set.update(sem_nums)


tile.TileContext._drain_and_barrier = _patched_drain_and_barrier


def split_excess_waits(nc, limit=1):
    for fn in nc.m.functions:
        for bb in fn.blocks:
            il = bb.instructions
            i = 0
            while i < len(il):
                inst = il[i]
                si = inst.sync_info
                if si is not None and len(si.on_wait) > limit:
                    waits = list(si.on_wait)
                    movable = [w for w in waits if "ge" in (w.wait_mode or "")]
                    pinned = [w for w in waits if w not in movable]
                    keep_n = max(0, limit - len(pinned))
                    if keep_n:
                        keep = pinned + movable[len(movable) - keep_n :]
                        extra = movable[: len(movable) - keep_n]
                    else:
                        keep, extra = pinned, movable
                    if not extra:
                        i += 1
                        continue
                    nops = []
                    for w in extra:
                        nop = mybir.InstNoOp(
                            name=nc.get_next_instruction_name(), ins=[], outs=[]
                        )
                        nop.engine = inst.engine
                        nop.sync_info = bass_rust.SyncInfo(on_wait=[w], on_update=[])
                        nops.append(nop)
                    inst.sync_info = bass_rust.SyncInfo(
                        on_wait=keep, on_update=list(si.on_update)
                    )
                    for j, nop in enumerate(nops):
                        il.insert(i + j, nop)
                    i += len(nops)
                i += 1


# ---------------------------------------------------------------------------
# Token blocks. bf16 runs the PE at full rate for any width, so block sizes
# are unconstrained; PSUM caps a block at 512 fp32 columns (one 2 KiB bank).
# ---------------------------------------------------------------------------


def _blocks_lead(cap):
    """Small lead block first (fast pipeline start), even 512-max after."""
    assert cap >= 512
    sizes = [256]
    rem = cap - 256
    k = -(-rem // 512)
    base, extra = divmod(rem, k)
    sizes += [base + (1 if i < extra else 0) for i in range(k)]
    return sizes


def _blocks_tail(cap):
    """Small tail block last (fast drain), even 512-max before."""
    assert cap >= 512
    rem = cap - 128
    k = -(-rem // 512)
    base, extra = divmod(rem, k)
    return [base + (1 if i < extra else 0) for i in range(k)] + [128]


def _chunks(c0, c1, step):
    out = []
    while c0 < c1:
        out.append((c0, min(c0 + step, c1)))
        c0 = out[-1][1]
    return out


def build_program(CA, CB):
    nc = bass.Bass("TRN2", target_bir_lowering=False, debug=False, num_devices=N_CORES)
    TOT = CA + CB
    # xP: host-packed [128, KC*TOT] bf16; token block (t0,tb) occupies columns
    # [KC*t0, KC*(t0+tb)) laid out [p, (kc t)]. yP likewise [p, (dt t)].
    # Weights host-prepacked bf16 [512, 2048]: row block m2 (of 4) holds
    # output tiles ft=2*m2, 2*m2+1 as [p, (ft2 kc c)] - consumption order,
    # 4 KiB DMA lines.
    xP = nc.declare_dram_parameter("xP", [128, KC * TOT], BF16, isOutput=False)
    w1a = nc.declare_dram_parameter("w1a", [512, 2048], BF16, isOutput=False)
    w2a = nc.declare_dram_parameter("w2a", [512, 2048], BF16, isOutput=False)
    w1b = nc.declare_dram_parameter("w1b", [512, 2048], BF16, isOutput=False)
    w2b = nc.declare_dram_parameter("w2b", [512, 2048], BF16, isOutput=False)
    yP = nc.declare_dram_parameter("yP", [128, DT * TOT], BF16, isOutput=True)

    w_r = {
        "w1a": w1a.rearrange("(m p) x -> m p x", p=128),
        "w2a": w2a.rearrange("(m p) x -> m p x", p=128),
        "w1b": w1b.rearrange("(m p) x -> m p x", p=128),
        "w2b": w2b.rearrange("(m p) x -> m p x", p=128),
    }

    # Global block list: slot A (big expert) then slot B (small expert).
    blocks = []
    t = 0
    for tb in _blocks_lead(CA):
        blocks.append((t, tb, "a"))
        t += tb
    for tb in _blocks_tail(CB):
        blocks.append((t, tb, "b"))
        t += tb
    assert t == TOT
    nb = len(blocks)

    with tile.TileContext(nc) as tc:
        with (
            tc.tile_pool(name="wpool", bufs=1) as wpool,
            tc.tile_pool(name="xpool", bufs=4) as xpool,
            tc.tile_pool(name="mpool", bufs=2) as mpool,
            tc.tile_pool(name="tpool", bufs=4) as tpool,
            tc.tile_pool(name="opool", bufs=2) as opool,
            tc.tile_pool(name="psum", bufs=4, space="PSUM") as psum_pool,
        ):
            w_sb = {
                k: wpool.tile([128, FT * D_MODEL], BF16, tag=k, name=k)
                for k in ("w1a", "w2a", "w1b", "w2b")
            }

            def emit_w(key, eng, chunks):
                sb = w_sb[key]
                for m in chunks:
                    eng.dma_start(sb[:, m * 2048 : (m + 1) * 2048], w_r[key][m])

            # Queue plan (two HW DGE queues: sync=SP, scalar=ACT; gpsimd
            # rides the software queue). Chunk order is matched to the
            # first block's chain deadlines so the PE chases weights as
            # little as possible: w1a c0-c2 on scalar, c3 + all x on sync,
            # w2a/w2b + most outputs on gpsimd, w1b on scalar later.
            emit_w("w1a", nc.scalar, [0, 1, 2])

            # x0/x1 issued up front: their doorbells must execute before
            # the relu stream occupies the scalar engine, and w1a c3 rides
            # sync between them (emitted before any chain that reads it).
            x_tiles = {}

            def emit_x(bi, engs):
                t0, tb, _slot = blocks[bi]
                x_sb = xpool.tile([128, KC * tb], BF16, tag="x", name=f"x{bi}")
                for j, (c0, c1) in enumerate(_chunks(0, KC * tb, 2048)):
                    engs[j % len(engs)].dma_start(
                        x_sb[:, c0:c1], xP[:, KC * t0 + c0 : KC * t0 + c1]
                    )
                x_tiles[bi] = x_sb

            emit_x(0, [nc.sync])
            emit_w("w1a", nc.sync, [3])
            emit_x(1, [nc.scalar, nc.sync])
            emit_w("w2a", nc.gpsimd, [0, 1, 2, 3])

            # Short warm-up: the PE clock ramps with sustained activity;
            # dependency-free matmuls cover the first weight/x DMA latency
            # so real work starts on an already-ramping clock.
            warm = wpool.tile([128, 512], BF16, tag="warm")
            nc.vector.memset(warm[:], 0.0)
            for _ in range(8):
                wp = psum_pool.tile([128, 512], F32, tag="ps", name="warmp")
                nc.tensor.matmul(wp[:], warm[:, :128], warm[:], start=True, stop=True)

            # Software-pipelined emission: the PE stream is in-order, so
            # emit L1(b+1) before L2(b) - the PE always has layer-1 work
            # while layer-2 weights / x blocks are still streaming.
            mids = {}

            def l1(bi):
                t0, tb, slot = blocks[bi]
                w1_sb = w_sb["w1" + slot]
                if bi in x_tiles:
                    x_sb = x_tiles.pop(bi)
                else:
                    x_sb = xpool.tile([128, KC * tb], BF16, tag="x", name=f"x{bi}")
                    for c0, c1 in _chunks(0, KC * tb, 2048):
                        nc.sync.dma_start(
                            x_sb[:, c0:c1], xP[:, KC * t0 + c0 : KC * t0 + c1]
                        )
                mid_sb = mpool.tile([128, FT * tb], BF16, tag="mid", name=f"mid{bi}")
                mids[bi] = mid_sb
                for ft in range(FT):
                    ps = psum_pool.tile([128, tb], F32, tag="ps", name=f"ps{bi}_{ft}")
                    for kc in range(KC):
                        nc.tensor.matmul(
                            ps[:],
                            w1_sb[
                                :,
                                ft * D_MODEL + kc * 128 : ft * D_MODEL + kc * 128 + 128,
                            ],
                            x_sb[:, kc * tb : (kc + 1) * tb],
                            start=(kc == 0),
                            stop=(kc == KC - 1),
                        )
                    tmp = tpool.tile([128, tb], BF16, tag="tmp", name=f"tmp{bi}_{ft}")
                    nc.scalar.activation(
                        tmp[:], ps[:], mybir.ActivationFunctionType.Relu
                    )
                    nc.vector.tensor_mul(
                        mid_sb[:, ft * tb : (ft + 1) * tb], tmp[:], tmp[:]
                    )

            def l2(bi):
                t0, tb, slot = blocks[bi]
                w2_sb = w_sb["w2" + slot]
                mid_sb = mids.pop(bi)
                o_sb = opool.tile([128, DT * tb], BF16, tag="o", name=f"o{bi}")
                for dt_ in range(DT):
                    ps2 = psum_pool.tile(
                        [128, tb], F32, tag="ps2", name=f"ps2{bi}_{dt_}", bufs=4
                    )
                    for fc in range(FT):
                        nc.tensor.matmul(
                            ps2[:],
                            w2_sb[
                                :, dt_ * D_FF + fc * 128 : dt_ * D_FF + fc * 128 + 128
                            ],
                            mid_sb[:, fc * tb : (fc + 1) * tb],
                            start=(fc == 0),
                            stop=(fc == FT - 1),
                        )
                    if dt_ % 2:
                        nc.vector.tensor_copy(o_sb[:, dt_ * tb : (dt_ + 1) * tb], ps2[:])
                    else:
                        nc.scalar.activation(
                            o_sb[:, dt_ * tb : (dt_ + 1) * tb],
                            ps2[:],
                            mybir.ActivationFunctionType.Copy,
                        )
                    if bi == nb - 1 and dt_ == 3:
                        # Drain the tail block in two halves so the final
                        # DMA is half the size.
                        for c0, c1 in _chunks(0, DT * tb // 2, 2048):
                            nc.sync.dma_start(
                                yP[:, DT * t0 + c0 : DT * t0 + c1], o_sb[:, c0:c1]
                            )
                o_eng = nc.sync if bi >= nb - 2 else nc.gpsimd
                lo = DT * tb // 2 if bi == nb - 1 else 0
                for c0, c1 in _chunks(lo, DT * tb, 2048):
                    o_eng.dma_start(yP[:, DT * t0 + c0 : DT * t0 + c1], o_sb[:, c0:c1])

            LA = 1  # mid tiles live LA+1 blocks -> mpool bufs = LA+1
            for step in range(nb + LA):
                if step == 1:
                    emit_w("w1b", nc.scalar, [0, 1, 2, 3])
                if step == 2:
                    emit_w("w2b", nc.gpsimd, [0, 1, 2, 3])
                if step < nb:
                    l1(step)
                if step >= LA:
                    l2(step - LA)

    split_excess_waits(nc, limit=1)
    return nc


_PROGRAM_CACHE = {}


def _get_program(CA, CB):
    if (CA, CB) not in _PROGRAM_CACHE:
        _PROGRAM_CACHE[(CA, CB)] = build_program(CA, CB)
    return _PROGRAM_CACHE[(CA, CB)]


# ---------------------------------------------------------------------------
# Host side: routing, dispatch, combine.
# ---------------------------------------------------------------------------


def _pack_blocked(aT, blocks, total):
    """[1024, total] feature-major -> [128, 8*total], each token block laid
    out [p, (g t)] so the device moves one contiguous chunk per block."""
    g = aT.shape[0] // 128
    out = np.zeros((128, g * total), aT.dtype)
    for t0, tb, _ in blocks:
        out[:, g * t0 : g * (t0 + tb)] = (
            aT[:, t0 : t0 + tb]
            .reshape(g, 128, tb)
            .transpose(1, 0, 2)
            .reshape(128, g * tb)
        )
    return out


def _unpack_blocked(aP, blocks, total):
    g = aP.shape[1] // total
    out = np.empty((g * 128, total), aP.dtype)
    for t0, tb, _ in blocks:
        blk = aP[:, g * t0 : g * (t0 + tb)].reshape(128, g, tb)
        out[:, t0 : t0 + tb] = blk.transpose(1, 0, 2).reshape(g * 128, tb)
    return out


def _prep_weight(w):
    """[k=1024, m=1024] -> bf16 [512, 2048]: rows (m2 p), cols (ft2 kc c)."""
    a = np.asarray(w, dtype=np.float32).astype(NPBF16)
    a = a.reshape(KC, 128, 4, 2, 128).transpose(2, 1, 3, 0, 4)
    return np.ascontiguousarray(a.reshape(512, 2048))


def _device_blocks(CA, CB):
    blocks = []
    t = 0
    for tb in _blocks_lead(CA):
        blocks.append((t, tb, "a"))
        t += tb
    for tb in _blocks_tail(CB):
        blocks.append((t, tb, "b"))
        t += tb
    return blocks, t


def kernel(x, Wr, W1, W2, _trace=False):
    x = np.asarray(x)
    Wr = np.asarray(Wr)
    W1 = np.asarray(W1)
    W2 = np.asarray(W2)
    B, T, C = x.shape
    N = B * T
    xf = np.ascontiguousarray(x.reshape(N, C), dtype=np.float32)

    # Router in float64 (matches jax f32 top_k selections; verified).
    logits = xf.astype(np.float64) @ Wr.astype(np.float64)
    logits -= logits.max(axis=-1, keepdims=True)
    p = np.exp(logits)
    p /= p.sum(axis=-1, keepdims=True)
    idx = np.argsort(-p, axis=-1, kind="stable")[:, :TOP_K]  # [N, K]
    wts = np.take_along_axis(p, idx, axis=-1)  # [N, K]

    # Dispatch list sorted by expert.
    flat_e = idx.ravel()
    order = np.argsort(flat_e, kind="stable")
    tok_of_pair = np.repeat(np.arange(N), TOP_K)[order]
    w_of_pair = wts.ravel()[order]
    counts = np.bincount(flat_e, minlength=NUM_EXPERTS)
    starts = np.concatenate([[0], np.cumsum(counts)[:-1]])

    # Pair big experts with small ones; each pair is served by two cores,
    # each core taking half of both experts' tokens.
    esort = np.argsort(-counts, kind="stable")
    pairs = [(int(esort[i]), int(esort[NUM_EXPERTS - 1 - i])) for i in range(4)]
    halves = {e: (int(counts[e]) + 1) // 2 for e in range(NUM_EXPERTS)}
    CA = max(256, -(-max(halves[a] for a, _ in pairs) // 4) * 4)
    CB = max(256, -(-max(halves[b] for _, b in pairs) // 4) * 4)
    blocks, TOT = _device_blocks(CA, CB)

    wprep = {}
    for e in range(NUM_EXPERTS):
        wprep[e] = (_prep_weight(W1[e]), _prep_weight(W2[e]))

    def _gather(e, lo, hi):
        s = int(starts[e])
        toks = tok_of_pair[s + lo : s + hi]
        ws = w_of_pair[s + lo : s + hi].astype(np.float32)
        return toks, xf[toks] * np.sqrt(ws)[:, None]

    in_maps = []
    core_meta = []  # (toksA, toksB) per core
    for eA, eB in pairs:
        cA, cB = int(counts[eA]), int(counts[eB])
        hA, hB = halves[eA], halves[eB]
        for half in range(2):
            loA, hiA = (0, hA) if half == 0 else (hA, cA)
            loB, hiB = (0, hB) if half == 0 else (hB, cB)
            toksA, xgA = _gather(eA, loA, hiA)
            toksB, xgB = _gather(eB, loB, hiB)
            xTe = np.zeros((C, TOT), NPBF16)
            xTe[:, : len(toksA)] = xgA.astype(NPBF16).T
            xTe[:, CA : CA + len(toksB)] = xgB.astype(NPBF16).T
            in_maps.append(
                {
                    "xP": _pack_blocked(xTe, blocks, TOT),
                    "w1a": wprep[eA][0],
                    "w2a": wprep[eA][1],
                    "w1b": wprep[eB][0],
                    "w2b": wprep[eB][1],
                }
            )
            core_meta.append((toksA, toksB))

    nc = _get_program(CA, CB)
    res = run_bass_kernel_spmd(nc, in_maps, core_ids=list(range(N_CORES)), trace=_trace)

    out = np.zeros((N, C), np.float32)
    for core, (toksA, toksB) in enumerate(core_meta):
        yT = _unpack_blocked(res.results[core]["yP"], blocks, TOT)
        if len(toksA):
            np.add.at(out, toksA, yT[:, : len(toksA)].T.astype(np.float32))
        if len(toksB):
            np.add.at(out, toksB, yT[:, CA : CA + len(toksB)].T.astype(np.float32))
    if _trace:
        kernel._last_exec_time_ns = res.exec_time_ns
    return out.reshape(B, T, C)
